# revision 13
# baseline (speedup 1.0000x reference)
"""Binary-weight 3x3 conv (depth-1 conv3d), 32ch -> 32ch, on trn2.

Forward pass of a BNN conv: effective weights are scale[o,i] * sign(w[o,i,kh,kw])
(the straight-through-estimator machinery in the reference only affects grads).
Kernel depth is 1, so this is a 2D 3x3 same-padded conv applied independently to
each of N*D = 8*16 = 128 images of shape [32, 160, 160].

Strategy (per core; batch dim sharded 1:1 onto 8 cores):
  - 16 d-slices per core, processed in 4 groups of 4 images.
  - Images live in SBUF zero-padded to 162 cols, on 32 channels = partitions
    [32r, 32r+32) for image r of the group.
  - PE runs in 32x32 tile-packing mode: tile (r, c) computes image r,
    pixel-segment c. 16 concurrent matmuls per tap, 9 taps accumulate in PSUM
    (tap shifts = free-axis offsets into the padded image).
  - float32r matmuls: 1 cycle/row at N>=256 (vs 4 for exact fp32).
  - PSUM evacuated to SBUF split between DVE and ACT, then one DMA per round
    writes [c, o, r, px] straight into the NCDHW output layout.
"""

import numpy as np

import concourse.bass as bass
import concourse.mybir as mybir
import concourse.tile as tile
from concourse import bacc
from concourse import bass_utils

C = 32          # in = out channels
KH = KW = 3

# full-problem dims
FULL_N, FULL_D, FULL_H, FULL_W = 8, 16, 160, 160


def build_conv(tc, out_ap, x_ap, w_ap, D, H, W, seg_rows, strip_rows, variant):
    """Emit the conv program for one core. x: [32, D, H, W], out: [32, D, H, W].

    variant "f32":   exact fp32 matmuls (4 cyc/row), w: [128, 288] f32.
    variant "bf16x3": x and w each split into bf16 hi+lo; accumulate
        w_hi*x_hi + w_hi*x_lo + w_lo*x_hi (error ~2^-18), w: [128, 2, 288] bf16.
    """
    nc = tc.nc
    f32 = mybir.dt.float32
    bf16 = mybir.dt.bfloat16
    mm_dt = f32 if variant == "f32" else bf16

    IPG = 4                      # images per group (row tiles)
    SEGS = 4                     # pixel segments per round (col tiles)
    NMM = seg_rows * W           # moving free size per matmul
    RPR = SEGS * seg_rows        # output rows per round
    assert H % RPR == 0 and strip_rows % RPR == 0 and H % strip_rows == 0
    assert D % IPG == 0
    NGRP = D // IPG
    T = H // RPR                 # rounds per image
    RPS = strip_rows // RPR      # rounds per strip
    NS = H // strip_rows         # strips per image
    XROWS = strip_rows + 2
    WP = W + 2
    assert NMM <= 512

    x_r = x_ap.rearrange("i (g r) h w -> g r i h w", g=NGRP, r=IPG)
    out_r = out_ap.rearrange(
        "o (g r) (t c sr) w -> g t c o r (sr w)",
        g=NGRP, r=IPG, t=T, c=SEGS, sr=seg_rows,
    )

    with (
        tc.tile_pool(name="wpool", bufs=1) as wpool,
        tc.tile_pool(name="xpool", bufs=2) as xpool,
        tc.tile_pool(name="stpool", bufs=3) as stpool,
        tc.tile_pool(name="pspool", bufs=2, space="PSUM") as pspool,
    ):
        if variant == "f32":
            w_sb = wpool.tile([128, KH * KW * C], f32, tag="w")
        else:
            w_sb = wpool.tile([128, 2, KH * KW * C], bf16, tag="w")
        nc.sync.dma_start(w_sb[:], w_ap[:])

        for g in range(NGRP):
            for s in range(NS):
                X32 = xpool.tile([128, XROWS, WP], f32, tag="X32")
                r0 = s * strip_rows
                # left/right zero pad columns (DMA never writes cols 0, WP-1)
                nc.gpsimd.memset(X32[:, :, 0:1], 0.0)
                nc.gpsimd.memset(X32[:, :, WP - 1 : WP], 0.0)
                lo, hi, dlo = r0 - 1, r0 + strip_rows + 1, 0
                if lo < 0:
                    nc.gpsimd.memset(X32[:, 0:1, :], 0.0)
                    lo, dlo = 0, 1
                if hi > H:
                    nc.gpsimd.memset(X32[:, XROWS - 1 : XROWS, :], 0.0)
                    hi = H
                nrows = hi - lo
                for r in range(IPG):
                    nc.sync.dma_start(
                        X32[32 * r : 32 * r + 32, dlo : dlo + nrows, 1 : 1 + W],
                        x_r[g][r][:, lo:hi, :],
                    )

                if variant == "f32":
                    # comp -> (weight slice index or None, moving buffer)
                    comps = [(None, X32)]
                else:
                    Xhi = xpool.tile([128, XROWS, WP], bf16, tag="Xhi")
                    Xlo = xpool.tile([128, XROWS, WP], bf16, tag="Xlo")
                    nc.scalar.copy(Xhi[:], X32[:])
                    nc.vector.tensor_sub(Xlo[:], X32[:], Xhi[:])
                    comps = [(0, Xhi), (0, Xlo), (1, Xhi)]

                for lt in range(RPS):
                    t = s * RPS + lt
                    ps = pspool.tile([128, SEGS, 512], f32, tag="ps")
                    for ci, (wi, XB) in enumerate(comps):
                        for tap in range(KH * KW):
                            kh, kw = divmod(tap, KW)
                            for r in range(IPG):
                                if wi is None:
                                    lhsT = w_sb[
                                        32 * r : 32 * r + 32, 32 * tap : 32 * tap + 32
                                    ]
                                else:
                                    lhsT = w_sb[
                                        32 * r : 32 * r + 32, wi,
                                        32 * tap : 32 * tap + 32,
                                    ]
                                for c in range(SEGS):
                                    j = lt * RPR + seg_rows * c
                                    rhs = XB[
                                        32 * r : 32 * r + 32,
                                        j + kh : j + kh + seg_rows,
                                        kw : kw + W,
                                    ]
                                    nc.tensor.matmul(
                                        ps[32 * c : 32 * c + 32, r, 0:NMM],
                                        lhsT,
                                        rhs,
                                        start=(ci == 0 and tap == 0),
                                        stop=(
                                            ci == len(comps) - 1
                                            and tap == KH * KW - 1
                                        ),
                                        tile_position=(32 * r, 32 * c),
                                    )
                    st = stpool.tile([128, SEGS, NMM], f32, tag="st")
                    nc.vector.tensor_copy(st[:, 0:2, :], ps[:, 0:2, 0:NMM])
                    nc.scalar.copy(st[:, 2:4, :], ps[:, 2:4, 0:NMM])
                    for r in range(IPG):
                        nc.sync.dma_start(out_r[g, t][:, :, r], st[:, r, :])


def build_module(n_cores=8, D=FULL_D, H=FULL_H, W=FULL_W, seg_rows=2,
                 strip_rows=40, variant="bf16x3"):
    nc = bacc.Bacc(
        "TRN2",
        target_bir_lowering=False,
        debug=False,
        num_devices=n_cores,
    )
    x_d = nc.dram_tensor("x", [C, D, H, W], mybir.dt.float32, kind="ExternalInput")
    if variant == "f32":
        w_d = nc.dram_tensor(
            "w", [128, KH * KW * C], mybir.dt.float32, kind="ExternalInput"
        )
    else:
        w_d = nc.dram_tensor(
            "w", [128, 2, KH * KW * C], mybir.dt.bfloat16, kind="ExternalInput"
        )
    out_d = nc.dram_tensor(
        "out", [C, D, H, W], mybir.dt.float32, kind="ExternalOutput"
    )
    with tile.TileContext(nc) as tc:
        build_conv(
            tc, out_d.ap(), x_d.ap(), w_d.ap(), D, H, W, seg_rows, strip_rows,
            variant,
        )
    nc.compile()
    return nc


def binarize_weights(weights, variant="bf16x3"):
    """Host-side: [32,32,1,3,3] fp32 -> packed replicated weight tile.
    w_packed[32r+i, 32*tap+o] = scale[o,i] * sign(w[o,i,kh,kw]), tap = kh*3+kw.
    f32: [128, 288] f32.  bf16x3: [128, 2, 288] bf16 (hi, lo split)."""
    w = np.asarray(weights, dtype=np.float32)
    scale = np.mean(np.abs(w), axis=(2, 3, 4), keepdims=True)
    bw = (scale * np.sign(w)).astype(np.float32)          # [o, i, 1, 3, 3]
    wt = bw[:, :, 0].transpose(1, 2, 3, 0).reshape(C, KH * KW * C)  # [i, tap*32+o]
    full = np.ascontiguousarray(np.tile(wt, (4, 1)))       # [128, 288] f32
    if variant == "f32":
        return full
    import ml_dtypes
    hi = full.astype(ml_dtypes.bfloat16)
    lo = (full - hi.astype(np.float32)).astype(ml_dtypes.bfloat16)
    return np.ascontiguousarray(np.stack([hi, lo], axis=1))  # [128, 2, 288] bf16


_NC_CACHE = {}


def _get_nc(key, **kwargs):
    if key not in _NC_CACHE:
        _NC_CACHE[key] = build_module(**kwargs)
    return _NC_CACHE[key]


def run(x, weights, trace=False, variant="bf16x3", seg_rows=2, strip_rows=40):
    x = np.ascontiguousarray(np.asarray(x, dtype=np.float32))
    n_cores = x.shape[0]
    key = (n_cores, variant, seg_rows, strip_rows)
    nc = _get_nc(
        key, n_cores=n_cores, seg_rows=seg_rows, strip_rows=strip_rows,
        variant=variant,
    )
    w_packed = binarize_weights(weights, variant)
    in_maps = [{"x": x[n], "w": w_packed} for n in range(n_cores)]
    res = bass_utils.run_bass_kernel_spmd(
        nc, in_maps, core_ids=list(range(n_cores)), trace=trace
    )
    out = np.stack([res.results[n]["out"] for n in range(n_cores)])
    return out, res


def kernel(x, weights):
    out, _ = run(x, weights)
    return out


# revision 18
# speedup vs baseline: 1.5024x; 1.5024x over previous
"""Binary-weight 3x3 conv (depth-1 conv3d), 32ch -> 32ch, on trn2.

Forward pass of a BNN conv: effective weights are scale[o,i] * sign(w[o,i,kh,kw])
(the straight-through-estimator machinery in the reference only affects grads).
Kernel depth is 1, so this is a 2D 3x3 same-padded conv applied independently to
each of N*D = 8*16 = 128 images of shape [32, 160, 160].

Strategy (per core; batch dim sharded 1:1 onto 8 cores):
  - 16 d-slices per core, processed in 4 groups of 4 images.
  - Images live in SBUF zero-padded to 162 cols, on 32 channels = partitions
    [32r, 32r+32) for image r of the group.
  - PE runs in 32x32 tile-packing mode: tile (r, c) computes image r,
    pixel-segment c. 16 concurrent matmuls per tap, 9 taps accumulate in PSUM
    (tap shifts = free-axis offsets into the padded image).
  - float32r matmuls: 1 cycle/row at N>=256 (vs 4 for exact fp32).
  - PSUM evacuated to SBUF split between DVE and ACT, then one DMA per round
    writes [c, o, r, px] straight into the NCDHW output layout.
"""

import numpy as np

import concourse.bass as bass
import concourse.mybir as mybir
import concourse.tile as tile
from concourse import bacc
from concourse import bass_utils

C = 32          # in = out channels
KH = KW = 3

# full-problem dims
FULL_N, FULL_D, FULL_H, FULL_W = 8, 16, 160, 160


def build_conv(tc, out_ap, x_ap, w_ap, D, H, W, seg_rows, strip_rows, variant):
    """Emit the conv program for one core. x: [32, D, H, W], out: [32, D, H, W].

    variant "f32":   exact fp32 matmuls (4 cyc/row), w: [128, 288] f32.
    variant "bf16x3": x and w each split into bf16 hi+lo; accumulate
        w_hi*x_hi + w_hi*x_lo + w_lo*x_hi (error ~2^-18), w: [128, 2, 288] bf16.
    """
    nc = tc.nc
    f32 = mybir.dt.float32
    bf16 = mybir.dt.bfloat16
    mm_dt = f32 if variant == "f32" else bf16

    IPG = 4                      # images per group (row tiles)
    SEGS = 4                     # pixel segments per round (col tiles)
    NMM = seg_rows * W           # moving free size per matmul
    RPR = SEGS * seg_rows        # output rows per round
    assert H % RPR == 0 and strip_rows % RPR == 0 and H % strip_rows == 0
    assert D % IPG == 0
    NGRP = D // IPG
    T = H // RPR                 # rounds per image
    RPS = strip_rows // RPR      # rounds per strip
    NS = H // strip_rows         # strips per image
    XROWS = strip_rows + 2
    WP = W + 2
    assert NMM <= 512

    # x_ap is host-prepadded: [D, C, H+2, W+2] with zero borders, so a strip
    # is one fully-contiguous DMA per partition (partition stride = (H+2)*(W+2)).
    x_r = x_ap.rearrange("(g p) hp wp -> g p (hp wp)", g=NGRP, p=IPG * C)
    out_r = out_ap.rearrange(
        "o (g r) (t c sr) w -> g t c o r (sr w)",
        g=NGRP, r=IPG, t=T, c=SEGS, sr=seg_rows,
    )

    with (
        tc.tile_pool(name="wpool", bufs=1) as wpool,
        tc.tile_pool(name="xpool", bufs=2) as xpool,
        tc.tile_pool(name="stpool", bufs=3) as stpool,
        tc.tile_pool(name="pspool", bufs=2, space="PSUM") as pspool,
    ):
        if variant == "f32":
            w_sb = wpool.tile([128, KH * KW * C], f32, tag="w")
        else:
            w_sb = wpool.tile([128, 2, KH * KW * C], bf16, tag="w")
        nc.sync.dma_start(w_sb[:], w_ap[:])

        for g in range(NGRP):
            for s in range(NS):
                X32 = xpool.tile([128, XROWS, WP], f32, tag="X32")
                r0 = s * strip_rows
                # padded rows [r0, r0+XROWS) of each image, one contiguous
                # run of XROWS*WP elements per partition
                nc.sync.dma_start(
                    X32[:].rearrange("p a b -> p (a b)"),
                    x_r[g][:, r0 * WP : (r0 + XROWS) * WP],
                )

                if variant == "f32":
                    # comp -> (weight slice index or None, moving buffer)
                    comps = [(None, X32)]
                else:
                    Xhi = xpool.tile([128, XROWS, WP], bf16, tag="Xhi")
                    Xlo = xpool.tile([128, XROWS, WP], bf16, tag="Xlo")
                    nc.scalar.copy(Xhi[:], X32[:])
                    nc.vector.tensor_sub(Xlo[:], X32[:], Xhi[:])
                    comps = [(0, Xhi), (0, Xlo), (1, Xhi)]

                for lt in range(RPS):
                    t = s * RPS + lt
                    ps = pspool.tile([128, SEGS, 512], f32, tag="ps")
                    for ci, (wi, XB) in enumerate(comps):
                        for tap in range(KH * KW):
                            kh, kw = divmod(tap, KW)
                            # r innermost: consecutive matmuls hit different
                            # PE row groups so their weight loads overlap
                            for c in range(SEGS):
                                for r in range(IPG):
                                    if wi is None:
                                        lhsT = w_sb[
                                            32 * r : 32 * r + 32,
                                            32 * tap : 32 * tap + 32,
                                        ]
                                    else:
                                        lhsT = w_sb[
                                            32 * r : 32 * r + 32, wi,
                                            32 * tap : 32 * tap + 32,
                                        ]
                                    j = lt * RPR + seg_rows * c
                                    rhs = XB[
                                        32 * r : 32 * r + 32,
                                        j + kh : j + kh + seg_rows,
                                        kw : kw + W,
                                    ]
                                    nc.tensor.matmul(
                                        ps[32 * c : 32 * c + 32, r, 0:NMM],
                                        lhsT,
                                        rhs,
                                        start=(ci == 0 and tap == 0),
                                        stop=(
                                            ci == len(comps) - 1
                                            and tap == KH * KW - 1
                                        ),
                                        tile_position=(32 * r, 32 * c),
                                    )
                    st = stpool.tile([128, SEGS, NMM], f32, tag="st")
                    nc.vector.tensor_copy(st[:, 0:2, :], ps[:, 0:2, 0:NMM])
                    nc.scalar.copy(st[:, 2:4, :], ps[:, 2:4, 0:NMM])
                    for r in range(IPG):
                        nc.sync.dma_start(out_r[g, t][:, :, r], st[:, r, :])


def build_module(n_cores=8, D=FULL_D, H=FULL_H, W=FULL_W, seg_rows=2,
                 strip_rows=None, variant="f32"):
    if strip_rows is None:
        strip_rows = 80 if variant == "f32" else 40
    nc = bacc.Bacc(
        "TRN2",
        target_bir_lowering=False,
        debug=False,
        num_devices=n_cores,
    )
    x_d = nc.dram_tensor(
        "x", [D * C, H + 2, W + 2], mybir.dt.float32, kind="ExternalInput"
    )
    if variant == "f32":
        w_d = nc.dram_tensor(
            "w", [128, KH * KW * C], mybir.dt.float32, kind="ExternalInput"
        )
    else:
        w_d = nc.dram_tensor(
            "w", [128, 2, KH * KW * C], mybir.dt.bfloat16, kind="ExternalInput"
        )
    out_d = nc.dram_tensor(
        "out", [C, D, H, W], mybir.dt.float32, kind="ExternalOutput"
    )
    with tile.TileContext(nc) as tc:
        build_conv(
            tc, out_d.ap(), x_d.ap(), w_d.ap(), D, H, W, seg_rows, strip_rows,
            variant,
        )
    nc.compile()
    return nc


def binarize_weights(weights, variant="bf16x3"):
    """Host-side: [32,32,1,3,3] fp32 -> packed replicated weight tile.
    w_packed[32r+i, 32*tap+o] = scale[o,i] * sign(w[o,i,kh,kw]), tap = kh*3+kw.
    f32: [128, 288] f32.  bf16x3: [128, 2, 288] bf16 (hi, lo split)."""
    w = np.asarray(weights, dtype=np.float32)
    scale = np.mean(np.abs(w), axis=(2, 3, 4), keepdims=True)
    bw = (scale * np.sign(w)).astype(np.float32)          # [o, i, 1, 3, 3]
    wt = bw[:, :, 0].transpose(1, 2, 3, 0).reshape(C, KH * KW * C)  # [i, tap*32+o]
    full = np.ascontiguousarray(np.tile(wt, (4, 1)))       # [128, 288] f32
    if variant == "f32":
        return full
    import ml_dtypes
    hi = full.astype(ml_dtypes.bfloat16)
    lo = (full - hi.astype(np.float32)).astype(ml_dtypes.bfloat16)
    return np.ascontiguousarray(np.stack([hi, lo], axis=1))  # [128, 2, 288] bf16


_NC_CACHE = {}


def _get_nc(key, **kwargs):
    if key not in _NC_CACHE:
        _NC_CACHE[key] = build_module(**kwargs)
    return _NC_CACHE[key]


def pad_input(x):
    """[N, C, D, H, W] f32 -> [N, D*C, H+2, W+2] zero-padded, d-major."""
    n, c, d, h, w = x.shape
    xp = np.zeros((n, d, c, h + 2, w + 2), dtype=np.float32)
    xp[:, :, :, 1 : h + 1, 1 : w + 1] = x.transpose(0, 2, 1, 3, 4)
    return xp.reshape(n, d * c, h + 2, w + 2)


def run(x, weights, trace=False, variant="f32", seg_rows=2, strip_rows=None):
    x = np.asarray(x, dtype=np.float32)
    n_cores = x.shape[0]
    key = (n_cores, variant, seg_rows, strip_rows)
    nc = _get_nc(
        key, n_cores=n_cores, seg_rows=seg_rows, strip_rows=strip_rows,
        variant=variant,
    )
    xp = pad_input(x)
    w_packed = binarize_weights(weights, variant)
    in_maps = [{"x": xp[n], "w": w_packed} for n in range(n_cores)]
    res = bass_utils.run_bass_kernel_spmd(
        nc, in_maps, core_ids=list(range(n_cores)), trace=trace
    )
    out = np.stack([res.results[n]["out"] for n in range(n_cores)])
    return out, res


def kernel(x, weights):
    out, _ = run(x, weights)
    return out


# revision 29
# speedup vs baseline: 2.2708x; 1.5115x over previous
"""Binary-weight 3x3 conv (depth-1 conv3d), 32ch -> 32ch, on trn2.

Forward pass of a BNN conv: effective weights are scale[o,i] * sign(w[o,i,kh,kw])
(the straight-through-estimator machinery in the reference only affects grads).
Kernel depth is 1, so this is a 2D 3x3 same-padded conv applied independently to
each of N*D = 8*16 = 128 images of shape [32, 160, 160].

Strategy (per core; batch dim sharded 1:1 onto 8 cores):
  - 16 d-slices per core, processed in 4 groups of 4 images.
  - Images live in SBUF zero-padded to 162 cols, on 32 channels = partitions
    [32r, 32r+32) for image r of the group.
  - PE runs in 32x32 tile-packing mode: tile (r, c) computes image r,
    pixel-segment c. 16 concurrent matmuls per tap, 9 taps accumulate in PSUM
    (tap shifts = free-axis offsets into the padded image).
  - float32r matmuls: 1 cycle/row at N>=256 (vs 4 for exact fp32).
  - PSUM evacuated to SBUF split between DVE and ACT, then one DMA per round
    writes [c, o, r, px] straight into the NCDHW output layout.
"""

import numpy as np

import concourse.bass as bass
import concourse.mybir as mybir
import concourse.tile as tile
from concourse import bacc
from concourse import bass_utils

C = 32          # in = out channels
KH = KW = 3

# full-problem dims
FULL_N, FULL_D, FULL_H, FULL_W = 8, 16, 160, 160


def build_conv(tc, out_ap, x_ap, w_ap, D, H, W, seg_rows, strip_rows, variant):
    """Emit the conv program for one core. x: [32, D, H, W], out: [32, D, H, W].

    variant "f32":   exact fp32 matmuls (4 cyc/row), w: [128, 288] f32.
    variant "bf16x3": x and w each split into bf16 hi+lo; accumulate
        w_hi*x_hi + w_hi*x_lo + w_lo*x_hi (error ~2^-18), w: [128, 2, 288] bf16.
    """
    nc = tc.nc
    f32 = mybir.dt.float32
    bf16 = mybir.dt.bfloat16
    mm_dt = f32 if variant == "f32" else bf16

    IPG = 4                      # images per group (row tiles)
    SEGS = 4                     # pixel segments per round (col tiles)
    NMM = seg_rows * W           # moving free size per matmul
    RPR = SEGS * seg_rows        # output rows per round
    assert D % IPG == 0
    NGRP = D // IPG
    WP = W + 2
    assert NMM <= 512

    # round origins (first output row of each round); if RPR doesn't divide
    # H, a final overlap round recomputes a few rows (identical values)
    origins = list(range(0, H - RPR + 1, RPR))
    covered = origins[-1] + RPR
    if covered < H:
        origins.append(H - RPR)
    new_from = {}
    for k, j0 in enumerate(origins):
        new_from[j0] = max(j0, origins[k - 1] + RPR) if k else j0
    rounds_per_strip = max(1, strip_rows // RPR)
    strips = [
        origins[i : i + rounds_per_strip]
        for i in range(0, len(origins), rounds_per_strip)
    ]
    XROWS = max(js[-1] + RPR + 2 - js[0] for js in strips)

    # x_ap is host-prepadded: [D, C, H+2, W+2] with zero borders, so a strip
    # is one fully-contiguous DMA per partition (partition stride = (H+2)*(W+2)).
    x_r = x_ap.rearrange("(g p) hp wp -> g p (hp wp)", g=NGRP, p=IPG * C)
    # [g] -> (o, r, h*w): row-sliced per round/seg at DMA time. o outermost
    # so the DGE splits each output DMA across all 16 SDMA engines (it
    # splits on the outermost dest dim; with 4 outermost it used only 4)
    out_v = out_ap.rearrange("o (g r) h w -> g o r (h w)", g=NGRP, r=IPG)

    with (
        tc.tile_pool(name="wpool", bufs=1) as wpool,
        tc.tile_pool(name="xpool", bufs=2) as xpool,
        tc.tile_pool(name="stpool", bufs=3) as stpool,
        tc.tile_pool(name="pspool", bufs=2, space="PSUM") as pspool,
    ):
        if variant == "f32":
            w_sb = wpool.tile([128, KH * KW * C], f32, tag="w")
        else:
            w_sb = wpool.tile([128, 2, KH * KW * C], bf16, tag="w")
        nc.sync.dma_start(w_sb[:], w_ap[:])

        for g in range(NGRP):
            for si, strip in enumerate(strips):
                X32 = xpool.tile([128, XROWS, WP], f32, tag="X32")
                r0 = strip[0]
                nrows = strip[-1] + RPR + 2 - r0
                # padded rows [r0, r0+nrows) of each image, one contiguous
                # run of nrows*WP elements per partition
                nc.sync.dma_start(
                    X32[:, 0:nrows, :].rearrange("p a b -> p (a b)"),
                    x_r[g][:, r0 * WP : (r0 + nrows) * WP],
                )

                if variant == "f32":
                    # comp -> (weight slice index or None, moving buffer)
                    comps = [(None, X32)]
                else:
                    Xhi = xpool.tile([128, XROWS, WP], bf16, tag="Xhi")
                    Xlo = xpool.tile([128, XROWS, WP], bf16, tag="Xlo")
                    nc.scalar.copy(Xhi[:, 0:nrows, :], X32[:, 0:nrows, :])
                    nc.vector.tensor_sub(
                        Xlo[:, 0:nrows, :], X32[:, 0:nrows, :],
                        Xhi[:, 0:nrows, :],
                    )
                    comps = [(0, Xhi), (0, Xlo), (1, Xhi)]

                for j0 in strip:
                    ps = pspool.tile([128, SEGS, 512], f32, tag="ps")
                    for ci, (wi, XB) in enumerate(comps):
                        for tap in range(KH * KW):
                            kh, kw = divmod(tap, KW)
                            # r innermost: consecutive matmuls hit different
                            # PE row groups so their weight loads overlap
                            for c in range(SEGS):
                                for r in range(IPG):
                                    if wi is None:
                                        lhsT = w_sb[
                                            32 * r : 32 * r + 32,
                                            32 * tap : 32 * tap + 32,
                                        ]
                                    else:
                                        lhsT = w_sb[
                                            32 * r : 32 * r + 32, wi,
                                            32 * tap : 32 * tap + 32,
                                        ]
                                    j = j0 - r0 + seg_rows * c
                                    rhs = XB[
                                        32 * r : 32 * r + 32,
                                        j + kh : j + kh + seg_rows,
                                        kw : kw + W,
                                    ]
                                    nc.tensor.matmul(
                                        ps[32 * c : 32 * c + 32, r, 0:NMM],
                                        lhsT,
                                        rhs,
                                        start=(ci == 0 and tap == 0),
                                        stop=(
                                            ci == len(comps) - 1
                                            and tap == KH * KW - 1
                                        ),
                                        tile_position=(32 * r, 32 * c),
                                    )
                    st = stpool.tile([128, SEGS, NMM], f32, tag="st")
                    nc.vector.tensor_copy(st[:, 0:2, :], ps[:, 0:2, 0:NMM])
                    nc.scalar.copy(st[:, 2:4, :], ps[:, 2:4, 0:NMM])
                    for c in range(SEGS):
                        # overlap (remainder) round: skip segs fully covered
                        # by the previous round
                        if j0 + seg_rows * (c + 1) <= new_from[j0]:
                            continue
                        eng = nc.sync if c % 2 == 0 else nc.scalar
                        lo_px = (j0 + seg_rows * c) * W
                        eng.dma_start(
                            out_v[g][:, :, lo_px : lo_px + NMM],
                            st[32 * c : 32 * c + 32, :, :],
                        )


def build_module(n_cores=8, D=FULL_D, H=FULL_H, W=FULL_W, seg_rows=2,
                 strip_rows=None, variant="f32"):
    if strip_rows is None:
        strip_rows = 80 if variant == "f32" else 40
    nc = bacc.Bacc(
        "TRN2",
        target_bir_lowering=False,
        debug=False,
        num_devices=n_cores,
    )
    x_d = nc.dram_tensor(
        "x", [D * C, H + 2, W + 2], mybir.dt.float32, kind="ExternalInput"
    )
    if variant == "f32":
        w_d = nc.dram_tensor(
            "w", [128, KH * KW * C], mybir.dt.float32, kind="ExternalInput"
        )
    else:
        w_d = nc.dram_tensor(
            "w", [128, 2, KH * KW * C], mybir.dt.bfloat16, kind="ExternalInput"
        )
    out_d = nc.dram_tensor(
        "out", [C, D, H, W], mybir.dt.float32, kind="ExternalOutput"
    )
    with tile.TileContext(nc) as tc:
        build_conv(
            tc, out_d.ap(), x_d.ap(), w_d.ap(), D, H, W, seg_rows, strip_rows,
            variant,
        )
    nc.compile()
    return nc


def binarize_weights(weights, variant="bf16x3"):
    """Host-side: [32,32,1,3,3] fp32 -> packed replicated weight tile.
    w_packed[32r+i, 32*tap+o] = scale[o,i] * sign(w[o,i,kh,kw]), tap = kh*3+kw.
    f32: [128, 288] f32.  bf16x3: [128, 2, 288] bf16 (hi, lo split)."""
    w = np.asarray(weights, dtype=np.float32)
    scale = np.mean(np.abs(w), axis=(2, 3, 4), keepdims=True)
    bw = (scale * np.sign(w)).astype(np.float32)          # [o, i, 1, 3, 3]
    wt = bw[:, :, 0].transpose(1, 2, 3, 0).reshape(C, KH * KW * C)  # [i, tap*32+o]
    full = np.ascontiguousarray(np.tile(wt, (4, 1)))       # [128, 288] f32
    if variant == "f32":
        return full
    import ml_dtypes
    hi = full.astype(ml_dtypes.bfloat16)
    lo = (full - hi.astype(np.float32)).astype(ml_dtypes.bfloat16)
    return np.ascontiguousarray(np.stack([hi, lo], axis=1))  # [128, 2, 288] bf16


_NC_CACHE = {}


def _get_nc(key, **kwargs):
    if key not in _NC_CACHE:
        _NC_CACHE[key] = build_module(**kwargs)
    return _NC_CACHE[key]


def pad_input(x):
    """[N, C, D, H, W] f32 -> [N, D*C, H+2, W+2] zero-padded, d-major."""
    n, c, d, h, w = x.shape
    xp = np.zeros((n, d, c, h + 2, w + 2), dtype=np.float32)
    xp[:, :, :, 1 : h + 1, 1 : w + 1] = x.transpose(0, 2, 1, 3, 4)
    return xp.reshape(n, d * c, h + 2, w + 2)


def run(x, weights, trace=False, variant="f32", seg_rows=2, strip_rows=None):
    x = np.asarray(x, dtype=np.float32)
    n_cores = x.shape[0]
    key = (n_cores, variant, seg_rows, strip_rows)
    nc = _get_nc(
        key, n_cores=n_cores, seg_rows=seg_rows, strip_rows=strip_rows,
        variant=variant,
    )
    xp = pad_input(x)
    w_packed = binarize_weights(weights, variant)
    in_maps = [{"x": xp[n], "w": w_packed} for n in range(n_cores)]
    res = bass_utils.run_bass_kernel_spmd(
        nc, in_maps, core_ids=list(range(n_cores)), trace=trace
    )
    out = np.stack([res.results[n]["out"] for n in range(n_cores)])
    return out, res


def kernel(x, weights):
    out, _ = run(x, weights)
    return out


# revision 32
# speedup vs baseline: 2.2759x; 1.0022x over previous
"""Binary-weight 3x3 conv (depth-1 conv3d), 32ch -> 32ch, on trn2.

Forward pass of a BNN conv: effective weights are scale[o,i] * sign(w[o,i,kh,kw])
(the straight-through-estimator machinery in the reference only affects grads).
Kernel depth is 1, so this is a 2D 3x3 same-padded conv applied independently to
each of N*D = 8*16 = 128 images of shape [32, 160, 160].

Strategy (per core; batch dim sharded 1:1 onto 8 cores):
  - 16 d-slices per core, processed in 4 groups of 4 images.
  - Images live in SBUF zero-padded to 162 cols, on 32 channels = partitions
    [32r, 32r+32) for image r of the group.
  - PE runs in 32x32 tile-packing mode: tile (r, c) computes image r,
    pixel-segment c. 16 concurrent matmuls per tap, 9 taps accumulate in PSUM
    (tap shifts = free-axis offsets into the padded image).
  - float32r matmuls: 1 cycle/row at N>=256 (vs 4 for exact fp32).
  - PSUM evacuated to SBUF split between DVE and ACT, then one DMA per round
    writes [c, o, r, px] straight into the NCDHW output layout.
"""

import numpy as np

import concourse.bass as bass
import concourse.mybir as mybir
import concourse.tile as tile
from concourse import bacc
from concourse import bass_utils

C = 32          # in = out channels
KH = KW = 3

# full-problem dims
FULL_N, FULL_D, FULL_H, FULL_W = 8, 16, 160, 160


def build_conv(tc, out_ap, x_ap, w_ap, D, H, W, seg_rows, strip_rows, variant):
    """Emit the conv program for one core. x: [32, D, H, W], out: [32, D, H, W].

    variant "f32":   exact fp32 matmuls (4 cyc/row), w: [128, 288] f32.
    variant "bf16x3": x and w each split into bf16 hi+lo; accumulate
        w_hi*x_hi + w_hi*x_lo + w_lo*x_hi (error ~2^-18), w: [128, 2, 288] bf16.
    """
    nc = tc.nc
    f32 = mybir.dt.float32
    bf16 = mybir.dt.bfloat16
    mm_dt = f32 if variant == "f32" else bf16

    IPG = 4                      # images per group (row tiles)
    SEGS = 4                     # pixel segments per round (col tiles)
    NMM = seg_rows * W           # moving free size per matmul
    RPR = SEGS * seg_rows        # output rows per round
    assert D % IPG == 0
    NGRP = D // IPG
    WP = W + 2
    assert NMM <= 512

    # round origins (first output row of each round); if RPR doesn't divide
    # H, a final overlap round recomputes a few rows (identical values)
    origins = list(range(0, H - RPR + 1, RPR))
    covered = origins[-1] + RPR
    if covered < H:
        origins.append(H - RPR)
    new_from = {}
    for k, j0 in enumerate(origins):
        new_from[j0] = max(j0, origins[k - 1] + RPR) if k else j0
    rounds_per_strip = max(1, strip_rows // RPR)
    strips = [
        origins[i : i + rounds_per_strip]
        for i in range(0, len(origins), rounds_per_strip)
    ]
    XROWS = max(js[-1] + RPR + 2 - js[0] for js in strips)

    # x_ap is host-prepadded: [D, C, H+2, W+2] with zero borders, so a strip
    # is one fully-contiguous DMA per partition (partition stride = (H+2)*(W+2)).
    x_r = x_ap.rearrange("(g p) hp wp -> g p (hp wp)", g=NGRP, p=IPG * C)
    # [g] -> (o, r, h*w): row-sliced per round/seg at DMA time. o outermost
    # so the DGE splits each output DMA across all 16 SDMA engines (it
    # splits on the outermost dest dim; with 4 outermost it used only 4)
    out_v = out_ap.rearrange("o (g r) h w -> g o r (h w)", g=NGRP, r=IPG)

    xbufs = 3 if variant == "f32" else 2
    with (
        tc.tile_pool(name="wpool", bufs=1) as wpool,
        tc.tile_pool(name="xpool", bufs=xbufs) as xpool,
        tc.tile_pool(name="stpool", bufs=3) as stpool,
        tc.tile_pool(name="pspool", bufs=2, space="PSUM") as pspool,
    ):
        if variant == "f32":
            w_sb = wpool.tile([128, KH * KW * C], f32, tag="w")
        else:
            w_sb = wpool.tile([128, 2, KH * KW * C], bf16, tag="w")
        nc.sync.dma_start(w_sb[:], w_ap[:])

        for g in range(NGRP):
            for si, strip in enumerate(strips):
                X32 = xpool.tile([128, XROWS, WP], f32, tag="X32")
                r0 = strip[0]
                nrows = strip[-1] + RPR + 2 - r0
                # padded rows [r0, r0+nrows) of each image, one contiguous
                # run of nrows*WP elements per partition
                nc.sync.dma_start(
                    X32[:, 0:nrows, :].rearrange("p a b -> p (a b)"),
                    x_r[g][:, r0 * WP : (r0 + nrows) * WP],
                )

                if variant == "f32":
                    # comp -> (weight slice index or None, moving buffer)
                    comps = [(None, X32)]
                else:
                    Xhi = xpool.tile([128, XROWS, WP], bf16, tag="Xhi")
                    Xlo = xpool.tile([128, XROWS, WP], bf16, tag="Xlo")
                    nc.scalar.copy(Xhi[:, 0:nrows, :], X32[:, 0:nrows, :])
                    nc.vector.tensor_sub(
                        Xlo[:, 0:nrows, :], X32[:, 0:nrows, :],
                        Xhi[:, 0:nrows, :],
                    )
                    comps = [(0, Xhi), (0, Xlo), (1, Xhi)]

                for j0 in strip:
                    ps = pspool.tile([128, SEGS, 512], f32, tag="ps")
                    for ci, (wi, XB) in enumerate(comps):
                        for tap in range(KH * KW):
                            kh, kw = divmod(tap, KW)
                            # r innermost: consecutive matmuls hit different
                            # PE row groups so their weight loads overlap
                            for c in range(SEGS):
                                for r in range(IPG):
                                    if wi is None:
                                        lhsT = w_sb[
                                            32 * r : 32 * r + 32,
                                            32 * tap : 32 * tap + 32,
                                        ]
                                    else:
                                        lhsT = w_sb[
                                            32 * r : 32 * r + 32, wi,
                                            32 * tap : 32 * tap + 32,
                                        ]
                                    j = j0 - r0 + seg_rows * c
                                    rhs = XB[
                                        32 * r : 32 * r + 32,
                                        j + kh : j + kh + seg_rows,
                                        kw : kw + W,
                                    ]
                                    nc.tensor.matmul(
                                        ps[32 * c : 32 * c + 32, r, 0:NMM],
                                        lhsT,
                                        rhs,
                                        start=(ci == 0 and tap == 0),
                                        stop=(
                                            ci == len(comps) - 1
                                            and tap == KH * KW - 1
                                        ),
                                        tile_position=(32 * r, 32 * c),
                                    )
                    st = stpool.tile([128, SEGS, NMM], f32, tag="st")
                    nc.vector.tensor_copy(st[:, 0:2, :], ps[:, 0:2, 0:NMM])
                    nc.scalar.copy(st[:, 2:4, :], ps[:, 2:4, 0:NMM])
                    for c in range(SEGS):
                        # overlap (remainder) round: skip segs fully covered
                        # by the previous round
                        if j0 + seg_rows * (c + 1) <= new_from[j0]:
                            continue
                        eng = nc.sync if c % 2 == 0 else nc.scalar
                        lo_px = (j0 + seg_rows * c) * W
                        eng.dma_start(
                            out_v[g][:, :, lo_px : lo_px + NMM],
                            st[32 * c : 32 * c + 32, :, :],
                        )


def build_module(n_cores=8, D=FULL_D, H=FULL_H, W=FULL_W, seg_rows=3,
                 strip_rows=None, variant="f32"):
    if strip_rows is None:
        strip_rows = 48 if variant == "f32" else 36
    nc = bacc.Bacc(
        "TRN2",
        target_bir_lowering=False,
        debug=False,
        num_devices=n_cores,
    )
    x_d = nc.dram_tensor(
        "x", [D * C, H + 2, W + 2], mybir.dt.float32, kind="ExternalInput"
    )
    if variant == "f32":
        w_d = nc.dram_tensor(
            "w", [128, KH * KW * C], mybir.dt.float32, kind="ExternalInput"
        )
    else:
        w_d = nc.dram_tensor(
            "w", [128, 2, KH * KW * C], mybir.dt.bfloat16, kind="ExternalInput"
        )
    out_d = nc.dram_tensor(
        "out", [C, D, H, W], mybir.dt.float32, kind="ExternalOutput"
    )
    with tile.TileContext(nc) as tc:
        build_conv(
            tc, out_d.ap(), x_d.ap(), w_d.ap(), D, H, W, seg_rows, strip_rows,
            variant,
        )
    nc.compile()
    return nc


def binarize_weights(weights, variant="bf16x3"):
    """Host-side: [32,32,1,3,3] fp32 -> packed replicated weight tile.
    w_packed[32r+i, 32*tap+o] = scale[o,i] * sign(w[o,i,kh,kw]), tap = kh*3+kw.
    f32: [128, 288] f32.  bf16x3: [128, 2, 288] bf16 (hi, lo split)."""
    w = np.asarray(weights, dtype=np.float32)
    scale = np.mean(np.abs(w), axis=(2, 3, 4), keepdims=True)
    bw = (scale * np.sign(w)).astype(np.float32)          # [o, i, 1, 3, 3]
    wt = bw[:, :, 0].transpose(1, 2, 3, 0).reshape(C, KH * KW * C)  # [i, tap*32+o]
    full = np.ascontiguousarray(np.tile(wt, (4, 1)))       # [128, 288] f32
    if variant == "f32":
        return full
    import ml_dtypes
    hi = full.astype(ml_dtypes.bfloat16)
    lo = (full - hi.astype(np.float32)).astype(ml_dtypes.bfloat16)
    return np.ascontiguousarray(np.stack([hi, lo], axis=1))  # [128, 2, 288] bf16


_NC_CACHE = {}


def _get_nc(key, **kwargs):
    if key not in _NC_CACHE:
        _NC_CACHE[key] = build_module(**kwargs)
    return _NC_CACHE[key]


def pad_input(x):
    """[N, C, D, H, W] f32 -> [N, D*C, H+2, W+2] zero-padded, d-major."""
    n, c, d, h, w = x.shape
    xp = np.zeros((n, d, c, h + 2, w + 2), dtype=np.float32)
    xp[:, :, :, 1 : h + 1, 1 : w + 1] = x.transpose(0, 2, 1, 3, 4)
    return xp.reshape(n, d * c, h + 2, w + 2)


def run(x, weights, trace=False, variant="f32", seg_rows=3, strip_rows=None):
    x = np.asarray(x, dtype=np.float32)
    n_cores = x.shape[0]
    key = (n_cores, variant, seg_rows, strip_rows)
    nc = _get_nc(
        key, n_cores=n_cores, seg_rows=seg_rows, strip_rows=strip_rows,
        variant=variant,
    )
    xp = pad_input(x)
    w_packed = binarize_weights(weights, variant)
    in_maps = [{"x": xp[n], "w": w_packed} for n in range(n_cores)]
    res = bass_utils.run_bass_kernel_spmd(
        nc, in_maps, core_ids=list(range(n_cores)), trace=trace
    )
    out = np.stack([res.results[n]["out"] for n in range(n_cores)])
    return out, res


def kernel(x, weights):
    out, _ = run(x, weights)
    return out


# revision 33
# speedup vs baseline: 2.4520x; 1.0774x over previous
"""Binary-weight 3x3 conv (depth-1 conv3d), 32ch -> 32ch, on trn2.

Forward pass of a BNN conv: effective weights are scale[o,i] * sign(w[o,i,kh,kw])
(the straight-through-estimator machinery in the reference only affects grads).
Kernel depth is 1, so this is a 2D 3x3 same-padded conv applied independently to
each of N*D = 8*16 = 128 images of shape [32, 160, 160].

Strategy (per core; batch dim sharded 1:1 onto 8 cores):
  - 16 d-slices per core, processed in 4 groups of 4 images.
  - Images live in SBUF zero-padded to 162 cols, on 32 channels = partitions
    [32r, 32r+32) for image r of the group.
  - PE runs in 32x32 tile-packing mode: tile (r, c) computes image r,
    pixel-segment c. 16 concurrent matmuls per tap, 9 taps accumulate in PSUM
    (tap shifts = free-axis offsets into the padded image).
  - float32r matmuls: 1 cycle/row at N>=256 (vs 4 for exact fp32).
  - PSUM evacuated to SBUF split between DVE and ACT, then one DMA per round
    writes [c, o, r, px] straight into the NCDHW output layout.
"""

import numpy as np

import concourse.bass as bass
import concourse.mybir as mybir
import concourse.tile as tile
from concourse import bacc
from concourse import bass_utils

C = 32          # in = out channels
KH = KW = 3

# full-problem dims
FULL_N, FULL_D, FULL_H, FULL_W = 8, 16, 160, 160


def build_conv(tc, out_ap, x_ap, w_ap, D, H, W, seg_rows, strip_rows, variant):
    """Emit the conv program for one core. x: [32, D, H, W], out: [32, D, H, W].

    variant "f32":   exact fp32 matmuls (4 cyc/row), w: [128, 288] f32.
    variant "bf16x3": x and w each split into bf16 hi+lo; accumulate
        w_hi*x_hi + w_hi*x_lo + w_lo*x_hi (error ~2^-18), w: [128, 2, 288] bf16.
    """
    nc = tc.nc
    f32 = mybir.dt.float32
    bf16 = mybir.dt.bfloat16
    mm_dt = f32 if variant == "f32" else bf16

    IPG = 4                      # images per group (row tiles)
    SEGS = 4                     # pixel segments per round (col tiles)
    NMM = seg_rows * W           # moving free size per matmul
    RPR = SEGS * seg_rows        # output rows per round
    assert D % IPG == 0
    NGRP = D // IPG
    WP = W + 2
    assert NMM <= 512

    # round origins (first output row of each round); if RPR doesn't divide
    # H, a final overlap round recomputes a few rows (identical values)
    origins = list(range(0, H - RPR + 1, RPR))
    covered = origins[-1] + RPR
    if covered < H:
        origins.append(H - RPR)
    new_from = {}
    for k, j0 in enumerate(origins):
        new_from[j0] = max(j0, origins[k - 1] + RPR) if k else j0
    rounds_per_strip = max(1, strip_rows // RPR)
    strips = [
        origins[i : i + rounds_per_strip]
        for i in range(0, len(origins), rounds_per_strip)
    ]
    XROWS = max(js[-1] + RPR + 2 - js[0] for js in strips)

    # x_ap is host-prepadded: [D, C, H+2, W+2] with zero borders, so a strip
    # is one fully-contiguous DMA per partition (partition stride = (H+2)*(W+2)).
    x_r = x_ap.rearrange("(g p) hp wp -> g p (hp wp)", g=NGRP, p=IPG * C)
    # [g] -> (o, r, h*w): row-sliced per round/seg at DMA time. o outermost
    # so the DGE splits each output DMA across all 16 SDMA engines (it
    # splits on the outermost dest dim; with 4 outermost it used only 4)
    out_v = out_ap.rearrange("o (g r) h w -> g o r (h w)", g=NGRP, r=IPG)

    xbytes = XROWS * WP * 4
    xbufs = 3 if (variant == "f32" and 3 * xbytes < 160 * 1024) else 2
    with (
        tc.tile_pool(name="wpool", bufs=1) as wpool,
        tc.tile_pool(name="xpool", bufs=xbufs) as xpool,
        tc.tile_pool(name="stpool", bufs=3) as stpool,
        tc.tile_pool(name="pspool", bufs=2, space="PSUM") as pspool,
    ):
        if variant == "f32":
            w_sb = wpool.tile([128, KH * KW * C], f32, tag="w")
        else:
            w_sb = wpool.tile([128, 2, KH * KW * C], bf16, tag="w")
        nc.sync.dma_start(w_sb[:], w_ap[:])

        for g in range(NGRP):
            for si, strip in enumerate(strips):
                X32 = xpool.tile([128, XROWS, WP], f32, tag="X32")
                r0 = strip[0]
                nrows = strip[-1] + RPR + 2 - r0
                # padded rows [r0, r0+nrows) of each image, one contiguous
                # run of nrows*WP elements per partition
                nc.sync.dma_start(
                    X32[:, 0:nrows, :].rearrange("p a b -> p (a b)"),
                    x_r[g][:, r0 * WP : (r0 + nrows) * WP],
                )

                if variant == "f32":
                    # comp -> (weight slice index or None, moving buffer)
                    comps = [(None, X32)]
                else:
                    Xhi = xpool.tile([128, XROWS, WP], bf16, tag="Xhi")
                    Xlo = xpool.tile([128, XROWS, WP], bf16, tag="Xlo")
                    nc.scalar.copy(Xhi[:, 0:nrows, :], X32[:, 0:nrows, :])
                    nc.vector.tensor_sub(
                        Xlo[:, 0:nrows, :], X32[:, 0:nrows, :],
                        Xhi[:, 0:nrows, :],
                    )
                    comps = [(0, Xhi), (0, Xlo), (1, Xhi)]

                for j0 in strip:
                    ps = pspool.tile([128, SEGS, 512], f32, tag="ps")
                    for ci, (wi, XB) in enumerate(comps):
                        for tap in range(KH * KW):
                            kh, kw = divmod(tap, KW)
                            # r innermost: consecutive matmuls hit different
                            # PE row groups so their weight loads overlap
                            for c in range(SEGS):
                                for r in range(IPG):
                                    if wi is None:
                                        lhsT = w_sb[
                                            32 * r : 32 * r + 32,
                                            32 * tap : 32 * tap + 32,
                                        ]
                                    else:
                                        lhsT = w_sb[
                                            32 * r : 32 * r + 32, wi,
                                            32 * tap : 32 * tap + 32,
                                        ]
                                    j = j0 - r0 + seg_rows * c
                                    rhs = XB[
                                        32 * r : 32 * r + 32,
                                        j + kh : j + kh + seg_rows,
                                        kw : kw + W,
                                    ]
                                    nc.tensor.matmul(
                                        ps[32 * c : 32 * c + 32, r, 0:NMM],
                                        lhsT,
                                        rhs,
                                        start=(ci == 0 and tap == 0),
                                        stop=(
                                            ci == len(comps) - 1
                                            and tap == KH * KW - 1
                                        ),
                                        tile_position=(32 * r, 32 * c),
                                    )
                    st = stpool.tile([128, SEGS, NMM], f32, tag="st")
                    nc.vector.tensor_copy(st[:, 0:2, :], ps[:, 0:2, 0:NMM])
                    nc.scalar.copy(st[:, 2:4, :], ps[:, 2:4, 0:NMM])
                    for c in range(SEGS):
                        # overlap (remainder) round: skip segs fully covered
                        # by the previous round
                        if j0 + seg_rows * (c + 1) <= new_from[j0]:
                            continue
                        eng = nc.sync if c % 2 == 0 else nc.scalar
                        lo_px = (j0 + seg_rows * c) * W
                        eng.dma_start(
                            out_v[g][:, :, lo_px : lo_px + NMM],
                            st[32 * c : 32 * c + 32, :, :],
                        )


def build_module(n_cores=8, D=FULL_D, H=FULL_H, W=FULL_W, seg_rows=3,
                 strip_rows=None, variant="f32"):
    if strip_rows is None:
        strip_rows = 48 if variant == "f32" else 36
    nc = bacc.Bacc(
        "TRN2",
        target_bir_lowering=False,
        debug=False,
        num_devices=n_cores,
    )
    x_d = nc.dram_tensor(
        "x", [D * C, H + 2, W + 2], mybir.dt.float32, kind="ExternalInput"
    )
    if variant == "f32":
        w_d = nc.dram_tensor(
            "w", [128, KH * KW * C], mybir.dt.float32, kind="ExternalInput"
        )
    else:
        w_d = nc.dram_tensor(
            "w", [128, 2, KH * KW * C], mybir.dt.bfloat16, kind="ExternalInput"
        )
    out_d = nc.dram_tensor(
        "out", [C, D, H, W], mybir.dt.float32, kind="ExternalOutput"
    )
    with tile.TileContext(nc) as tc:
        build_conv(
            tc, out_d.ap(), x_d.ap(), w_d.ap(), D, H, W, seg_rows, strip_rows,
            variant,
        )
    nc.compile()
    return nc


def binarize_weights(weights, variant="bf16x3"):
    """Host-side: [32,32,1,3,3] fp32 -> packed replicated weight tile.
    w_packed[32r+i, 32*tap+o] = scale[o,i] * sign(w[o,i,kh,kw]), tap = kh*3+kw.
    f32: [128, 288] f32.  bf16x3: [128, 2, 288] bf16 (hi, lo split)."""
    w = np.asarray(weights, dtype=np.float32)
    scale = np.mean(np.abs(w), axis=(2, 3, 4), keepdims=True)
    bw = (scale * np.sign(w)).astype(np.float32)          # [o, i, 1, 3, 3]
    wt = bw[:, :, 0].transpose(1, 2, 3, 0).reshape(C, KH * KW * C)  # [i, tap*32+o]
    full = np.ascontiguousarray(np.tile(wt, (4, 1)))       # [128, 288] f32
    if variant == "f32":
        return full
    import ml_dtypes
    hi = full.astype(ml_dtypes.bfloat16)
    lo = (full - hi.astype(np.float32)).astype(ml_dtypes.bfloat16)
    return np.ascontiguousarray(np.stack([hi, lo], axis=1))  # [128, 2, 288] bf16


_NC_CACHE = {}


def _get_nc(key, **kwargs):
    if key not in _NC_CACHE:
        _NC_CACHE[key] = build_module(**kwargs)
    return _NC_CACHE[key]


def pad_input(x):
    """[N, C, D, H, W] f32 -> [N, D*C, H+2, W+2] zero-padded, d-major."""
    n, c, d, h, w = x.shape
    xp = np.zeros((n, d, c, h + 2, w + 2), dtype=np.float32)
    xp[:, :, :, 1 : h + 1, 1 : w + 1] = x.transpose(0, 2, 1, 3, 4)
    return xp.reshape(n, d * c, h + 2, w + 2)


def run(x, weights, trace=False, variant="f32", seg_rows=3, strip_rows=None):
    x = np.asarray(x, dtype=np.float32)
    n_cores = x.shape[0]
    key = (n_cores, variant, seg_rows, strip_rows)
    nc = _get_nc(
        key, n_cores=n_cores, seg_rows=seg_rows, strip_rows=strip_rows,
        variant=variant,
    )
    xp = pad_input(x)
    w_packed = binarize_weights(weights, variant)
    in_maps = [{"x": xp[n], "w": w_packed} for n in range(n_cores)]
    res = bass_utils.run_bass_kernel_spmd(
        nc, in_maps, core_ids=list(range(n_cores)), trace=trace
    )
    out = np.stack([res.results[n]["out"] for n in range(n_cores)])
    return out, res


def kernel(x, weights):
    out, _ = run(x, weights)
    return out
